# revision 16
# baseline (speedup 1.0000x reference)
"""Bass/Trainium2 kernel for bidirectional Chamfer loss.

Problem: y_true [8, 8192, 3], y_pred [8, 8192, 3] fp32 ->
  scalar = mean_b(sum_n min_m d2[b,n,m]) + mean_b(sum_m min_n d2[b,n,m])
  with d2 = max(|x|^2 + |y|^2 - 2 x.y, 0).

Strategy:
  - Data-parallel over batch: 8 batches -> 8 NeuronCores (1 each).
  - Per core, two matmul passes over the [8192, 8192] distance matrix:
    pass A tiles rows of x on partitions (row-min), pass B tiles rows of y
    (col-min). The full d2 expression is folded into a single K=24
    contraction of bf16 "triple-split" operands (hi/mid/lo bf16 limbs give
    ~fp32 product accuracy while running the PE at its 1-cycle/column bf16
    rate; fp32 matmul would run 4x slower).
  - Each [128, 512] PSUM tile is min-reduced on the DVE over 4-bank
    [128, 2048] groups with a running column-min; per 128-row block the
    per-partition min lands in a [128, 64] SBUF tile, DMA'd out.
  - Host epilogue: relu + sums in fp64, mean over batch.
"""

import numpy as np
import ml_dtypes

N = 8192  # points per cloud
D = 3
K = 24  # contraction lanes of the augmented matmul
PART = 128  # partition block (rows of the distance matrix per tile)
FREE = 512  # matmul moving free dim (one PSUM bank of fp32)
GROUP = 4  # matmuls per PSUM group (4 banks, reduced by one DVE op)
NB = N // PART  # 64 row blocks
NG = N // (FREE * GROUP)  # 4 column groups per row block

_BF16 = ml_dtypes.bfloat16


def _split3(a):
    """fp32 -> three bf16 limbs with a ~= hi+mid+lo to ~2^-24 relative."""
    a = np.ascontiguousarray(a, np.float32)
    hi = a.astype(_BF16)
    r1 = a - hi.astype(np.float32)
    mid = r1.astype(_BF16)
    r2 = r1 - mid.astype(np.float32)
    lo = r2.astype(_BF16)
    return hi, mid, lo


def _build_sides(stat, mov, stat_sq, mov_sq):
    """Build [K, N] bf16 stationary (lhsT) / moving (rhs) lane matrices.

    lane i contributes A[i, n] * B[i, m] to PSUM[n, m]; the 24 lanes sum to
    stat_sq[n] + mov_sq[m] - 2 * stat[n].mov[m] at ~fp32 accuracy.
    """
    A = np.zeros((K, stat.shape[0]), _BF16)
    B = np.zeros((K, mov.shape[0]), _BF16)
    t = (-2.0 * mov.astype(np.float64)).astype(np.float32)
    for d in range(D):
        xh, xm, xl = _split3(stat[:, d])
        th, tm, tl = _split3(t[:, d])
        r = 6 * d
        A[r + 0], B[r + 0] = xh, th
        A[r + 1], B[r + 1] = xh, tm
        A[r + 2], B[r + 2] = xm, th
        A[r + 3], B[r + 3] = xm, tm
        A[r + 4], B[r + 4] = xh, tl
        A[r + 5], B[r + 5] = xl, th
    sh, sm, sl = _split3(mov_sq)
    A[18:21] = _BF16(1.0)
    B[18], B[19], B[20] = sh, sm, sl
    qh, qm, ql = _split3(stat_sq)
    A[21], A[22], A[23] = qh, qm, ql
    B[21:24] = _BF16(1.0)
    return A, B


_NC_CACHE = {}


def _build_bass(repeat=1, act_groups=0):
    """Trace + schedule the per-core Bass program (two fused min passes).

    repeat > 1 re-runs the whole compute body that many times (idempotent —
    same mins every iteration); used only for wall-clock timing rigs.

    act_groups: of the NG=4 column groups per row block, how many are
    drained via ScalarE (PSUM -> fp16 SBUF copy, then one DVE
    tensor_tensor_reduce over the two halves). The rest are min-reduced by
    the DVE straight from PSUM. Splitting the PSUM drain between both
    engines is what buys parallelism: DVE reduce ~2.3us vs ACT copy ~1.9us
    + DVE TTR ~1.1us per [128, 2048] group.
    """
    key = (repeat, act_groups)
    if key in _NC_CACHE:
        return _NC_CACHE[key]

    from concourse import bacc, mybir
    import concourse.tile as tile

    nc = bacc.Bacc("TRN2", target_bir_lowering=False, debug=False)
    f32 = mybir.dt.float32
    bf16 = mybir.dt.bfloat16

    ins = {
        name: nc.dram_tensor(name, [K, N], bf16, kind="ExternalInput")
        for name in ("afwd", "bfwd", "abwd", "bbwd")
    }
    outs = {
        name: nc.dram_tensor(name, [PART, NB], f32, kind="ExternalOutput")
        for name in ("fwdmin", "bwdmin")
    }

    f16 = mybir.dt.float16
    GF = GROUP * FREE

    def tt_min(out_ap, a_ap, b_ap):
        """DVE elementwise min via InstTensorTensor (2x mode for fp16 SBUF
        step-1 operands; bass has no wrapper for the plain TT opcode)."""
        eng = nc.vector
        return eng.add_instruction(
            mybir.InstTensorTensor(
                name=nc.get_next_instruction_name(),
                op=mybir.AluOpType.min,
                ins=[eng.lower_ap(a_ap), eng.lower_ap(b_ap)],
                outs=[eng.lower_ap(out_ap)],
            )
        )

    with tile.TileContext(nc) as tc:
        with (
            tc.tile_pool(name="lanes", bufs=1) as lanes,
            tc.tile_pool(name="psum", bufs=2, space="PSUM") as psum,
            tc.tile_pool(name="conv", bufs=5) as conv_pool,
            tc.tile_pool(name="junk", bufs=2) as junk_pool,
            tc.tile_pool(name="colmin", bufs=3) as colmin_pool,
            tc.tile_pool(name="mins", bufs=1) as mins_pool,
        ):
            lane_tiles = {}
            for name in ("afwd", "bfwd", "abwd", "bbwd"):
                lane_t = lanes.tile([K, N], bf16, tag=name)
                nc.sync.dma_start(out=lane_t[:], in_=ins[name][:])
                lane_tiles[name] = lane_t
            mins_tiles = {}
            for name in ("fwdmin", "bwdmin"):
                mins_t = mins_pool.tile([PART, NB], f32, tag=name)
                mins_tiles[name] = mins_t
            for rep in range(repeat):
              for pass_name, a_name, b_name, out_name in (
                ("fwd", "afwd", "bfwd", "fwdmin"),
                ("bwd", "abwd", "bbwd", "bwdmin"),
              ):
                a_sb = lane_tiles[a_name]
                b_sb = lane_tiles[b_name]
                mins_sb = mins_tiles[out_name]
                for nb in range(NB):
                    lhsT = a_sb[:, nb * PART : (nb + 1) * PART]
                    n_cm = (NG - act_groups) + (1 if act_groups else 0)
                    cm = colmin_pool.tile([PART, n_cm], f32)
                    cvs = []
                    for g in range(NG):
                        ps = psum.tile([PART, GF], f32)
                        for k in range(GROUP):
                            c0 = (g * GROUP + k) * FREE
                            nc.tensor.matmul(
                                ps[:, k * FREE : (k + 1) * FREE],
                                lhsT,
                                b_sb[:, c0 : c0 + FREE],
                                start=True,
                                stop=True,
                            )
                        if g < act_groups:
                            cv = conv_pool.tile([PART, GF], f16)
                            nc.scalar.copy(out=cv[:], in_=ps[:])
                            cvs.append(cv)
                        else:
                            nc.vector.tensor_reduce(
                                out=cm[:, g - act_groups : g - act_groups + 1],
                                in_=ps[:],
                                axis=mybir.AxisListType.X,
                                op=mybir.AluOpType.min,
                            )
                    if act_groups:
                        # chain full-width TT-mins across the converted
                        # groups, then halve twice and reduce
                        u = cvs[0]
                        for cv_next in cvs[1:]:
                            un = junk_pool.tile([PART, GF], f16, tag="uc")
                            tt_min(un[:], u[:], cv_next[:])
                            u = un
                        w = GF
                        while w > FREE:
                            un = junk_pool.tile(
                                [PART, w // 2], f16, tag=f"uh{w}"
                            )
                            tt_min(un[:], u[:, : w // 2], u[:, w // 2 :])
                            u = un
                            w //= 2
                        nc.vector.tensor_reduce(
                            out=cm[:, n_cm - 1 : n_cm],
                            in_=u[:],
                            axis=mybir.AxisListType.X,
                            op=mybir.AluOpType.min,
                        )
                    nc.vector.tensor_reduce(
                        out=mins_sb[:, nb : nb + 1],
                        in_=cm[:],
                        axis=mybir.AxisListType.X,
                        op=mybir.AluOpType.min,
                    )
                nc.sync.dma_start(out=outs[out_name][:], in_=mins_sb[:])

    nc.compile()
    _NC_CACHE[repeat] = nc
    return nc


def kernel(y_true: np.ndarray, y_pred: np.ndarray) -> np.ndarray:
    import os

    from concourse import bass_utils

    act_groups = int(os.environ.get("CHAMFER_ACT_GROUPS", "0"))

    x = np.asarray(y_true, np.float32)
    y = np.asarray(y_pred, np.float32)
    B = x.shape[0]

    in_maps = []
    for b in range(B):
        xb, yb = x[b], y[b]
        x2 = (xb.astype(np.float64) ** 2).sum(1).astype(np.float32)
        y2 = (yb.astype(np.float64) ** 2).sum(1).astype(np.float32)
        afwd, bfwd = _build_sides(xb, yb, x2, y2)
        abwd, bbwd = _build_sides(yb, xb, y2, x2)
        in_maps.append(
            {"afwd": afwd, "bfwd": bfwd, "abwd": abwd, "bbwd": bbwd}
        )

    nc = _build_bass(act_groups=act_groups)
    results = bass_utils.run_bass_kernel_spmd(
        nc, in_maps, core_ids=list(range(B))
    ).results

    total_fwd = 0.0
    total_bwd = 0.0
    for b in range(B):
        fwd = np.maximum(results[b]["fwdmin"].astype(np.float64), 0.0)
        bwd = np.maximum(results[b]["bwdmin"].astype(np.float64), 0.0)
        total_fwd += fwd.sum()
        total_bwd += bwd.sum()
    return np.float32(total_fwd / B + total_bwd / B)
